# revision 1
# baseline (speedup 1.0000x reference)
import sys
sys.path.insert(0, "/opt/trn_rl_repo")
import numpy as np
import ml_dtypes
from contextlib import ExitStack

import concourse.bass as bass
import concourse.bacc as bacc
import concourse.tile as tile
from concourse import mybir
from concourse.bass_utils import run_bass_kernel_spmd

F32 = mybir.dt.float32
F32R = mybir.dt.float32r
FP8 = mybir.dt.float8e4
BF16 = mybir.dt.bfloat16
AF = mybir.ActivationFunctionType
ALU = mybir.AluOpType

B, NQ, NK, DV, H, D = 4, 1024, 1024, 1024, 16, 64
TS = 512            # tokens per core in the output phase
HD = 512            # head-dims per core (8 heads)
EPS = 1e-5
SCALE = 1.0 / 32.0  # 1/sqrt(DV)
RG = [[0, 1], [2, 3], [4, 5], [6, 7]]

_CACHE = {}


def _build():
    nc = bacc.Bacc("TRN2", target_bir_lowering=False)

    qt_d = nc.dram_tensor("qt", [DV, NQ], BF16, kind="ExternalInput")
    kt_d = nc.dram_tensor("kt", [DV, NK], BF16, kind="ExternalInput")
    qres_d = nc.dram_tensor("qres", [DV, TS], BF16, kind="ExternalInput")
    wq_d = nc.dram_tensor("wq", [DV, HD], BF16, kind="ExternalInput")
    wk_d = nc.dram_tensor("wk", [DV, HD], BF16, kind="ExternalInput")
    wv_d = nc.dram_tensor("wv", [DV, HD], BF16, kind="ExternalInput")
    wo_d = nc.dram_tensor("wo", [DV, DV], BF16, kind="ExternalInput")
    ucbq_d = nc.dram_tensor("ucbq", [2, HD], BF16, kind="ExternalInput")
    ucbk_d = nc.dram_tensor("ucbk", [2, HD], BF16, kind="ExternalInput")
    ucbv_d = nc.dram_tensor("ucbv", [2, HD], BF16, kind="ExternalInput")
    ucbo_d = nc.dram_tensor("ucbo", [2, DV], BF16, kind="ExternalInput")
    msel_d = nc.dram_tensor("msel", [128, 4], F32, kind="ExternalInput")
    ones_d = nc.dram_tensor("ones1", [128, 1], F32, kind="ExternalInput")
    out_d = nc.dram_tensor("out", [DV, TS], F32, kind="ExternalOutput")

    with tile.TileContext(nc) as tc, ExitStack() as ctx:
        ctx.enter_context(nc.allow_low_precision(reason="bf16 pipeline by design"))
        P = ctx.enter_context
        pmain = P(tc.tile_pool(name="main", bufs=1))
        pw = P(tc.tile_pool(name="w", bufs=3))
        pbig = P(tc.tile_pool(name="big", bufs=1))
        pa = P(tc.tile_pool(name="a", bufs=2))
        pout = P(tc.tile_pool(name="out", bufs=2))
        psq = P(tc.tile_pool(name="sq", bufs=2))
        prow = P(tc.tile_pool(name="row", bufs=3))
        pgen = P(tc.tile_pool(name="gen", bufs=2, space="PSUM"))
        patt = P(tc.tile_pool(name="att", bufs=2, space="PSUM"))
        pctx = P(tc.tile_pool(name="ctx", bufs=2, space="PSUM"))
        pdram = P(tc.tile_pool(name="dram", bufs=1, space="DRAM"))

        # ---------------- input DMAs ----------------
        skt = pbig.tile([128, 8 * NK], BF16, tag="big", name="skt")
        for h_ in range(2):
            nc.sync.dma_start(
                skt[:].rearrange("p (c t) -> p c t", t=NK)[:, :, h_ * 512:(h_ + 1) * 512],
                kt_d[:].rearrange("(c p) t -> p c t", p=128)[:, :, h_ * 512:(h_ + 1) * 512])
        wv_sb = pw.tile([128, 4096], BF16, tag="w", name="wv_sb")
        nc.sync.dma_start(
            wv_sb[:].rearrange("p (c n) -> p c n", n=HD),
            wv_d[:].rearrange("(c p) n -> p c n", p=128))
        sqt = pmain.tile([128, 8 * NQ], BF16, tag="sqt")
        for h_ in range(2):
            nc.sync.dma_start(
                sqt[:].rearrange("p (c t) -> p c t", t=NQ)[:, :, h_ * 512:(h_ + 1) * 512],
                qt_d[:].rearrange("(c p) t -> p c t", p=128)[:, :, h_ * 512:(h_ + 1) * 512])
        wk_sb = pw.tile([128, 4096], BF16, tag="w", name="wk_sb")
        nc.sync.dma_start(
            wk_sb[:].rearrange("p (c n) -> p c n", n=HD),
            wk_d[:].rearrange("(c p) n -> p c n", p=128))
        wq_sb = pw.tile([128, 4096], BF16, tag="w", name="wq_sb")
        nc.sync.dma_start(
            wq_sb[:].rearrange("p (c n) -> p c n", n=HD),
            wq_d[:].rearrange("(c p) n -> p c n", p=128))
        ucbq = pmain.tile([2, HD], BF16, tag="ucbq")
        nc.sync.dma_start(ucbq[:], ucbq_d[:])
        ucbk = pmain.tile([2, HD], BF16, tag="ucbk")
        nc.sync.dma_start(ucbk[:], ucbk_d[:])
        ucbv = pmain.tile([2, HD], BF16, tag="ucbv")
        nc.sync.dma_start(ucbv[:], ucbv_d[:])
        ucbo = pmain.tile([2, DV], BF16, tag="ucbo")
        nc.sync.dma_start(ucbo[:], ucbo_d[:])
        sqres = pmain.tile([128, 4096], BF16, tag="sqres")
        nc.sync.dma_start(
            sqres[:].rearrange("p (c t) -> p c t", t=TS),
            qres_d[:].rearrange("(c p) t -> p c t", p=128))
        msel = pmain.tile([128, 4], F32, tag="msel")
        nc.sync.dma_start(msel[:], msel_d[:])

        # ---------------- constants ----------------
        ones_col = pmain.tile([128, 1], BF16, tag="ones_col")
        nc.vector.memset(ones_col[:], 1.0)
        ones_r = pmain.tile([128, 1], F32R, tag="ones_r")
        nc.sync.dma_start(ones_r[:], ones_d[:].bitcast(F32R))
        ones_row = pmain.tile([1, 128], BF16, tag="ones_row")
        nc.vector.memset(ones_row[:], 1.0)
        epst = pmain.tile([1, 1], F32, tag="epst")
        nc.vector.memset(epst[:], EPS)
        warm = pmain.tile([1, 1], F32, tag="warm")
        nc.scalar.activation(warm[:], epst[:], AF.Sqrt)

        # persistent row tiles
        mq = pmain.tile([1, NQ], F32, tag="mq")
        mk = pmain.tile([1, NK], F32, tag="mk")
        mo = pmain.tile([1, TS], F32, tag="mo")
        iq = pmain.tile([1, NQ], BF16, tag="iq")
        ik = pmain.tile([1, NK], BF16, tag="ik")
        io = pmain.tile([1, TS], BF16, tag="io")
        rq2 = pmain.tile([2, NQ], BF16, tag="rq2")
        nc.vector.memset(rq2[:], 1.0)
        rk2 = pmain.tile([2, NK], BF16, tag="rk2")
        nc.vector.memset(rk2[:], 1.0)
        ro2 = pmain.tile([2, TS], BF16, tag="ro2")
        nc.vector.memset(ro2[:], 1.0)
        bi_q = pmain.tile([128, NQ], BF16, tag="bi_q")
        bi_k = pmain.tile([128, NK], BF16, tag="bi_k")
        bi_o = pmain.tile([128, TS], BF16, tag="bi_o")

        def stats(xsb, toks, m_row, i_row, r2, s_row=None, nm_row=None):
            # per-token mean (f32), inv-std (bf16), and r2 row0 = -(m*i) from
            # the raw feature-major tiles xsb [128, 8*toks]
            for t_ in range(toks // 512):
                s0 = pgen.tile([128, 512], F32, tag="gp")
                s1 = pgen.tile([128, 512], F32, tag="gp")
                is_r = xsb.dtype != BF16
                for fc in range(8):
                    xs = xsb[:, fc * toks + t_ * 512: fc * toks + t_ * 512 + 512]
                    sq = psq.tile([128, 512], BF16, tag="sq")
                    nc.gpsimd.tensor_mul(sq[:], xs, xs)
                    if is_r:
                        nc.tensor.matmul(s0[0:1, :], ones_r[:], xs,
                                         start=(fc == 0), stop=(fc == 7))
                    else:
                        nc.tensor.matmul(s0[0:1, :], ones_col[:], xs,
                                         start=(fc == 0), stop=(fc == 7))
                    nc.tensor.matmul(s1[0:1, :], ones_col[:], sq[:],
                                     start=(fc == 0), stop=(fc == 7))
                sl = slice(t_ * 512, t_ * 512 + 512)
                nc.vector.tensor_scalar_mul(m_row[0:1, sl], s0[0:1, :], 1.0 / DV)
                m2 = prow.tile([1, 512], F32, tag="r")
                nc.vector.tensor_mul(m2[:], m_row[0:1, sl], m_row[0:1, sl])
                var = prow.tile([1, 512], F32, tag="r")
                nc.vector.scalar_tensor_tensor(var[:], s1[0:1, :], 1.0 / DV, m2[:],
                                               op0=ALU.mult, op1=ALU.subtract)
                srow = prow.tile([1, 512], F32, tag="r")
                nc.scalar.activation(srow[:], var[:], AF.Sqrt, bias=epst[:])
                i32 = prow.tile([1, 512], F32, tag="r")
                nc.vector.reciprocal(i32[:], srow[:])
                nc.vector.tensor_copy(i_row[0:1, sl], i32[:])
                nc.vector.scalar_tensor_tensor(r2[0:1, sl], m_row[0:1, sl], -1.0,
                                               i32[:], op0=ALU.mult, op1=ALU.mult)
                if s_row is not None:
                    nc.vector.tensor_copy(s_row[0:1, sl], srow[:])
                if nm_row is not None:
                    nc.vector.tensor_scalar_mul(nm_row[0:1, sl], m_row[0:1, sl], -1.0)

        def bcast(i_row, toks, dst):
            # dst [128, toks] bf16 = broadcast of i_row over partitions
            for t_ in range(toks // 512):
                bb = pgen.tile([128, 512], F32, tag="gp")
                nc.tensor.matmul(bb[:], ones_row[:],
                                 i_row[0:1, t_ * 512: t_ * 512 + 512],
                                 start=True, stop=True)
                nc.vector.tensor_copy(dst[:, t_ * 512: t_ * 512 + 512], bb[:])

        qp = pmain.tile([128, 4096], BF16, tag="qp")   # [2head-dims, hp*1024+t]
        kp = pmain.tile([128, 4096], BF16, tag="kp")
        vp = pmain.tile([128, 8 * 520], BF16, tag="vp")  # 65-slot head layout
        nc.vector.memset(
            vp[:].rearrange("p (s e) -> p s e", e=65)[:, :, 64:65], 1.0)

        stats(skt, NK, mk, ik, rk2)
        bcast(ik, NK, bi_k)
        stats(sqt, NQ, mq, iq, rq2)
        bcast(iq, NQ, bi_q)
        nc.scalar.activation(warm[:], epst[:], AF.Exp)

        # x' = x * inv_std (in place; LN mean folded into rank-1 correction)
        for fc in range(8):
            eng = nc.gpsimd if fc % 2 else nc.vector
            eng.tensor_mul(skt[:, fc * NK:(fc + 1) * NK],
                           skt[:, fc * NK:(fc + 1) * NK], bi_k[:])

        # Vp
        for kt_ in range(8):
            ps = pgen.tile([128, 512], F32, tag="gp")
            for fc in range(8):
                nc.tensor.matmul(ps[:],
                                 skt[:, fc * NK + kt_ * 128: fc * NK + kt_ * 128 + 128],
                                 wv_sb[:, fc * HD:(fc + 1) * HD],
                                 start=(fc == 0), stop=False)
            nc.tensor.matmul(ps[:], rk2[:, kt_ * 128: kt_ * 128 + 128], ucbv[:],
                             start=False, stop=True)
            dst = vp[:, kt_ * 520: kt_ * 520 + 520].rearrange(
                "p (s e) -> p s e", e=65)[:, :, 0:64]
            nc.vector.tensor_copy(dst, ps[:].rearrange("p (s e) -> p s e", e=64))

        for fc in range(8):
            eng = nc.gpsimd if fc % 2 else nc.vector
            eng.tensor_mul(sqt[:, fc * NQ:(fc + 1) * NQ],
                           sqt[:, fc * NQ:(fc + 1) * NQ], bi_q[:])


        # wo reuses skt's slot (dead after Kp/Vp)
        wo_sb = pbig.tile([128, 8 * NK], BF16, tag="big", name="wo_sb")
        nc.sync.dma_start(
            wo_sb[:].rearrange("p (c n) -> p c n", n=1024),
            wo_d[:].rearrange("(c p) n -> p c n", p=128))

        # ---------------- attention ----------------
        # phase 1: partner-token halves (qc=1) per head pair; each head pair's
        # ctx ships through the pair AllGather as soon as it is ready.
        # phase 2: local-token halves (qc=0), merged into O afterwards.
        o_remote = pmain.tile([128, 2048], FP8, tag="o_remote")  # [dims, hp*512+q]
        o_local = pmain.tile([128, 2048], BF16, tag="o_local")
        ex_sb = pmain.tile([128, 4096], FP8, tag="ex_sb")    # [j*2048+hp*512+q]
        osb = pmain.tile([128, 4096], F32R, tag="osb")        # O, global fc-major
        SH4 = 128 * 512

        def qkt_head_pair(hp, qc, a_e, a_o):
            # scores + exp for both heads of pair hp, query chunk qc
            for kcp in range(4):
                pse = patt.tile([128, 1024], F32, tag="att")
                pso = patt.tile([128, 1024], F32, tag="att")
                for half in range(2):
                    kc = 2 * kcp + half
                    nc.tensor.matmul(
                        pse[:, half * 512: half * 512 + 512],
                        kp[0:64, hp * 1024 + kc * 128: hp * 1024 + kc * 128 + 128],
                        qp[0:64, hp * 1024 + qc * 512: hp * 1024 + qc * 512 + 512],
                        start=True, stop=True)
                    nc.tensor.matmul(
                        pso[:, half * 512: half * 512 + 512],
                        kp[64:128, hp * 1024 + kc * 128: hp * 1024 + kc * 128 + 128],
                        qp[64:128, hp * 1024 + qc * 512: hp * 1024 + qc * 512 + 512],
                        start=True, stop=True)
                nc.scalar.activation(a_e[:, kcp * 1024: kcp * 1024 + 1024],
                                     pse[:], AF.Exp, scale=SCALE)
                nc.scalar.activation(a_o[:, kcp * 1024: kcp * 1024 + 1024],
                                     pso[:], AF.Exp, scale=SCALE)

        def ctx_head(hp, half, a_t, dst):
            # dst: [64, 512] slice of o_remote/o_local for this head
            hl = 2 * hp + half
            cs = pctx.tile([128, 512], F32, tag="ctx")
            for kc in range(8):
                nc.tensor.matmul(
                    cs[0:65, :],
                    vp[:, kc * 520 + hl * 65: kc * 520 + hl * 65 + 65],
                    a_t[:, kc * 512: kc * 512 + 512],
                    start=(kc == 0), stop=(kc == 7))
            rr = prow.tile([1, 512], BF16, tag="rr")
            nc.vector.reciprocal(rr[:], cs[64:65, :])
            pb = pgen.tile([128, 512], F32, tag="gp")
            nc.tensor.matmul(pb[0:64, :], ones_row[0:1, 0:64], rr[:],
                             start=True, stop=True)
            bs = psq.tile([64, 512], BF16, tag="bs")
            nc.vector.tensor_copy(bs[:], pb[0:64, :])
            nc.vector.tensor_mul(dst, cs[0:64, :], bs[:])

        # ---- phase 1: Kp + Qp + partner halves + exchange ----
        for hp in range(4):
            for t_ in range(2):
                ps = pgen.tile([128, 512], F32, tag="gp")
                for fc in range(8):
                    nc.tensor.matmul(
                        ps[:],
                        wk_sb[:, fc * HD + hp * 128: fc * HD + hp * 128 + 128],
                        skt[:, fc * NK + t_ * 512: fc * NK + t_ * 512 + 512],
                        start=(fc == 0), stop=False)
                nc.tensor.matmul(ps[:], ucbk[:, hp * 128: hp * 128 + 128],
                                 rk2[:, t_ * 512: t_ * 512 + 512],
                                 start=False, stop=True)
                if t_ == 0:
                    nc.vector.tensor_copy(
                        kp[:, hp * 1024 + t_ * 512: hp * 1024 + t_ * 512 + 512], ps[:])
                else:
                    nc.scalar.activation(
                        kp[:, hp * 1024 + t_ * 512: hp * 1024 + t_ * 512 + 512],
                        ps[:], AF.Copy)
            for t_ in range(2):
                ps = pgen.tile([128, 512], F32, tag="gp")
                for fc in range(8):
                    nc.tensor.matmul(
                        ps[:],
                        wq_sb[:, fc * HD + hp * 128: fc * HD + hp * 128 + 128],
                        sqt[:, fc * NQ + t_ * 512: fc * NQ + t_ * 512 + 512],
                        start=(fc == 0), stop=False)
                nc.tensor.matmul(ps[:], ucbq[:, hp * 128: hp * 128 + 128],
                                 rq2[:, t_ * 512: t_ * 512 + 512],
                                 start=False, stop=True)
                if t_ == 0:
                    nc.vector.tensor_copy(
                        qp[:, hp * 1024 + t_ * 512: hp * 1024 + t_ * 512 + 512], ps[:])
                else:
                    nc.scalar.activation(
                        qp[:, hp * 1024 + t_ * 512: hp * 1024 + t_ * 512 + 512],
                        ps[:], AF.Copy)

            a_e = pa.tile([128, 4096], BF16, tag="a", name=f"ae1_{hp}")
            a_o = pa.tile([128, 4096], BF16, tag="a", name=f"ao1_{hp}")
            qkt_head_pair(hp, 1, a_e, a_o)
            for half, a_t in ((0, a_e), (1, a_o)):
                ctx_head(hp, half, a_t,
                         o_remote[half * 64: half * 64 + 64,
                                  hp * 512: hp * 512 + 512])

            exi = pdram.tile([SH4], FP8, tag=f"exin{hp}", name=f"exin{hp}")
            nc.sync.dma_start(
                exi[:].rearrange("(p q) -> p q", p=128),
                o_remote[:, hp * 512: hp * 512 + 512])
            exo = pdram.tile([2 * SH4], FP8, tag=f"exout{hp}", name=f"exout{hp}")
            nc.gpsimd.collective_compute(
                "AllGather", ALU.bypass, replica_groups=RG,
                ins=[exi[:]], outs=[exo[:]])
            for j in range(2):
                nc.gpsimd.dma_start(
                    ex_sb[:, j * 2048 + hp * 512: j * 2048 + hp * 512 + 512],
                    exo[j * SH4:(j + 1) * SH4].rearrange("(p q) -> p q", p=128))
                # partial merge: osb = qres + masked remote ctx
                col = (j * 4 + hp) * 512
                t1 = psq.tile([128, 512], BF16, tag="mrg")
                nc.gpsimd.tensor_scalar_mul(
                    t1[:], ex_sb[:, j * 2048 + hp * 512: j * 2048 + hp * 512 + 512],
                    msel[:, j:j + 1])
                nc.gpsimd.tensor_add(osb[:, col:col + 512],
                                     sqres[:, col:col + 512], t1[:])

        # ---- phase 2: local halves + local merge ----
        for hp in range(4):
            a_e = pa.tile([128, 4096], BF16, tag="a", name=f"ae0_{hp}")
            a_o = pa.tile([128, 4096], BF16, tag="a", name=f"ao0_{hp}")
            qkt_head_pair(hp, 0, a_e, a_o)
            for half, a_t in ((0, a_e), (1, a_o)):
                ctx_head(hp, half, a_t,
                         o_local[half * 64: half * 64 + 64,
                                 hp * 512: hp * 512 + 512])
            for j in range(2):
                col = (j * 4 + hp) * 512
                t2 = psq.tile([128, 512], BF16, tag="mrg")
                nc.gpsimd.tensor_scalar_mul(
                    t2[:], o_local[:, hp * 512: hp * 512 + 512],
                    msel[:, 2 + j:3 + j])
                nc.gpsimd.tensor_add(osb[:, col:col + 512],
                                     osb[:, col:col + 512], t2[:])


        stats(osb, TS, mo, io, ro2)
        bcast(io, TS, bi_o)
        onorm = pmain.tile([128, 4096], BF16, tag="onorm")
        for fc in range(8):
            eng = nc.gpsimd if fc % 2 else nc.vector
            eng.tensor_mul(onorm[:, fc * TS:(fc + 1) * TS],
                           osb[:, fc * TS:(fc + 1) * TS], bi_o[:])

        for m in range(8):
            ps = pgen.tile([128, 512], F32, tag="gp")
            for fc in range(8):
                nc.tensor.matmul(
                    ps[:],
                    wo_sb[:, fc * 1024 + m * 128: fc * 1024 + m * 128 + 128],
                    onorm[:, fc * TS:(fc + 1) * TS],
                    start=(fc == 0), stop=False)
            nc.tensor.matmul(ps[:], ucbo[:, m * 128: m * 128 + 128], ro2[:],
                             start=False, stop=True)
            ro = pout.tile([128, TS], BF16, tag="ro")
            nc.scalar.activation(ro[:], ps[:], AF.Relu)
            ob = pout.tile([128, TS], F32, tag="ob")
            nc.gpsimd.tensor_add(ob[:], ro[:], osb[:, m * TS:(m + 1) * TS])
            nc.sync.dma_start(out_d[m * 128:(m + 1) * 128, :], ob[:])

    nc.compile()
    return nc


def _prep_in_maps(inputs):
    Q = np.asarray(inputs["Q"], np.float32)
    K = np.asarray(inputs["K"], np.float32)
    wq, bq = np.asarray(inputs["wq"], np.float32), np.asarray(inputs["bq"], np.float32)
    wk, bk = np.asarray(inputs["wk"], np.float32), np.asarray(inputs["bk"], np.float32)
    wv, bv = np.asarray(inputs["wv"], np.float32), np.asarray(inputs["bv"], np.float32)
    wo, bo = np.asarray(inputs["wo"], np.float32), np.asarray(inputs["bo"], np.float32)
    gq, betaq = np.asarray(inputs["gq"], np.float32), np.asarray(inputs["betaq"], np.float32)
    gk, betak = np.asarray(inputs["gk"], np.float32), np.asarray(inputs["betak"], np.float32)
    g0, beta0 = np.asarray(inputs["g0"], np.float32), np.asarray(inputs["beta0"], np.float32)

    BF = ml_dtypes.bfloat16
    wqf = gq[:, None] * wq
    wkf = gk[:, None] * wk
    wvf = gk[:, None] * wv
    wof = (g0[:, None] * wo).astype(BF)
    bq_row = betaq @ wq + bq
    bk_row = betak @ wk + bk
    bv_row = betak @ wv + bv
    bo_row = beta0 @ wo + bo
    u_o = wof.astype(np.float32).sum(axis=0)
    ucbo = np.ascontiguousarray(np.stack([u_o, bo_row]).astype(BF))

    in_maps = []
    for c in range(8):
        b, hh = c // 2, c % 2
        hsl = slice(hh * HD, (hh + 1) * HD)
        tsl = slice(hh * TS, (hh + 1) * TS)
        osl = slice((1 - hh) * TS, (2 - hh) * TS)
        wq_c = np.ascontiguousarray(wqf[:, hsl].astype(BF))
        wk_c = np.ascontiguousarray(wkf[:, hsl].astype(BF))
        wv_c = np.ascontiguousarray(wvf[:, hsl].astype(BF))
        # local query order: my token half first, partner's second
        qloc = np.concatenate([Q[b, tsl, :], Q[b, osl, :]], axis=0)
        msel = np.zeros((128, 4), np.float32)
        msel[:, 1 - hh] = 1.0   # take partner's AllGather chunk for their half
        msel[:, 2 + hh] = 1.0   # take my local ctx for my half
        m = {
            "qt": np.ascontiguousarray(qloc.T.astype(BF)),
            "kt": np.ascontiguousarray(K[b].T.astype(BF)),
            "qres": np.ascontiguousarray(Q[b, tsl, :].T.astype(BF)),
            "wq": wq_c, "wk": wk_c, "wv": wv_c, "wo": wof,
            "ucbq": np.ascontiguousarray(
                np.stack([wq_c.astype(np.float32).sum(0), bq_row[hsl]]).astype(BF)),
            "ucbk": np.ascontiguousarray(
                np.stack([wk_c.astype(np.float32).sum(0), bk_row[hsl]]).astype(BF)),
            "ucbv": np.ascontiguousarray(
                np.stack([wv_c.astype(np.float32).sum(0), bv_row[hsl]]).astype(BF)),
            "ucbo": ucbo,
            "msel": msel,
            "ones1": np.ones((128, 1), np.float32),
        }
        in_maps.append(m)
    return in_maps


def kernel(**inputs):
    if "nc" not in _CACHE:
        _CACHE["nc"] = _build()
    nc = _CACHE["nc"]
    in_maps = _prep_in_maps(inputs)
    _CACHE["in_map0"] = in_maps[0]
    trace = _CACHE.get("trace", False)
    res = run_bass_kernel_spmd(nc, in_maps, list(range(8)), trace=trace)
    _CACHE["last"] = res

    out = np.empty((B, NQ, DV), np.float32)
    for c in range(8):
        b, hh = c // 2, c % 2
        tsl = slice(hh * TS, (hh + 1) * TS)
        out[b, tsl, :] = res.results[c]["out"].T
    return out

